# revision 10
# baseline (speedup 1.0000x reference)
"""PWC-Net correlation (nn_CorrBlock) Trainium2 Bass kernel — col-tiled Gram.

Problem: feat1, feat2 [8, 256, 80, 160] f32 -> leaky_relu(corr, 0.1)
  corr[n, d, h, w] = (1/256) * sum_c feat1[n,c,h,w] * feat2p[n,c,h+dy,w+dx]
  d = 9*dy + dx, (dy, dx) in [0..8]^2, feat2p zero-padded by 4.

Strategy (data-parallel, 1 sample per NeuronCore, 8 cores):
  - Cast-load both feature maps to SBUF as bf16 (feat2 zero-padded to
    [88 x 168]), channels on partitions (2 chunks of 128); feat1 is
    pre-scaled in place by 1/256 (exact exponent shift in bf16).
  - Supertile = 8x16 pixels, split into 4 subtiles of 4x8 = 32 pixels.
    Each subtile's taps form a 12x16 = 192-tap patch of feat2p.
  - PE column tiling: subtile g's matmuls target PSUM partitions
    [32g, 32g+32) -> tile_position=(0, 32g), so the four M=32, N=192
    matmuls (per C-chunk) run CONCURRENTLY on the four 32-column strips
    of the PE array (each with its own XBUS stream). 2 C-chunks
    accumulate in PSUM. Effective PE cost ~2x192 cycles per 128 pixels
    vs 2x384 for the full-array 128-pixel Gram.
  - PSUM: T4 tile [128, 4*512] f32 = 4 banks, one supertile per bank
    (192 of 512 cols used), pool bufs=2 -> 8 banks in flight.
  - Drain+leaky-relu PSUM->SBUF bf16, split ACT (Prelu, first SA cols)
    / DVE (scalar_tensor_tensor max(0.1x, x), rest), grouped over the
    4 supertiles of a T4 via 3D APs. V rows are 256-col padded so the
    store has 512B-aligned contiguous runs.
  - One batched DMA per supertile row stores [128, 10*256] bf16,
    alternating between the SP and Pool DGE queues.
  - Host extracts each pixel's 9x9 window from its subtile's 12x16
    patch (pure gather) and casts to f32.
"""

import sys

sys.path.insert(0, "/opt/trn_rl_repo")
import numpy as np

N, C, H, W = 8, 256, 80, 160
HP, WP = 88, 168  # padded feat2 dims (+4 each side)
TH, TW = 8, 16  # supertile pixels
SH, SW = 4, 8  # subtile pixels (one PE column strip)
PH, PW = SH + 8, SW + 8  # tap patch per subtile: 12 x 16
PATCH = PH * PW  # 192
VROW = 256  # per-pixel stored cols (512B bf16, 192 data + 64 pad)
BANK = 512  # PSUM bank stride (f32)
NTI, NTJ = H // TH, W // TW  # 10 x 10 supertiles
SA = 128  # ACT drain columns (Prelu); DVE copy-drains PATCH-SA then stt-relus
G, NG = 10, 8  # h-group size for load interleaving
SCALE = 1.0 / C

_cache = {}


def _build(repeat=1, use_prelu=True):
    import concourse.tile as tile
    from concourse import bacc, mybir
    from concourse.ap import AP

    F32, BF16 = mybir.dt.float32, mybir.dt.bfloat16
    nc = bacc.Bacc("TRN2", target_bir_lowering=False, debug=False)
    f1 = nc.dram_tensor("f1", [C, H * W], F32, kind="ExternalInput")
    f2 = nc.dram_tensor("f2", [C, H * W], F32, kind="ExternalInput")
    O = nc.dram_tensor("O", [NTI * NTJ * 128, VROW], BF16, kind="ExternalOutput")

    with tile.TileContext(nc) as tc:
        with (
            tc.tile_pool(name="inp", bufs=1) as inp,
            tc.tile_pool(name="work", bufs=2) as work,
            tc.tile_pool(name="ps", bufs=2, space="PSUM") as ps,
        ):
            f1sb, f2sb = [], []
            for cc in range(2):
                t1 = inp.tile([128, H * W], BF16, tag=f"f1_{cc}")
                f1sb.append(t1)
                t2 = inp.tile([128, HP * WP], BF16, tag=f"f2_{cc}")
                a = t2[:]
                pp = a.ap[0][0]
                # zero pads: top 4 rows, bottom 4 rows, left pad of row 4,
                # then fused right(h)+left(h+1) pads of the 80 data rows
                nc.vector.memset(t2[:, 0 : 4 * WP], 0.0)
                nc.vector.memset(t2[:, 84 * WP : 88 * WP], 0.0)
                nc.vector.memset(t2[:, 4 * WP : 4 * WP + 4], 0.0)
                lr = AP(a.tensor, a.offset + 4 * WP + 164, [[pp, 128], [WP, 80], [1, 8]])
                nc.vector.memset(lr, 0.0)
                f2sb.append(t2)
            # loads emitted interleaved in consumer (h-group) order so the
            # first tile rows' working set arrives before later pieces
            for g in range(NG + 1):
                for cc in range(2):
                    a = f2sb[cc][:]
                    pp = a.ap[0][0]
                    src2 = f2.ap()[128 * cc : 128 * (cc + 1), :].rearrange(
                        "c (h w) -> c h w", h=H
                    )
                    hp_lo, hp_hi = 10 * g, min(10 * g + 10, HP)
                    d_lo, d_hi = max(hp_lo, 4), min(hp_hi, 84)
                    if d_lo < d_hi:
                        dst = AP(
                            a.tensor,
                            a.offset + d_lo * WP + 4,
                            [[pp, 128], [WP, d_hi - d_lo], [1, W]],
                        )
                        nc.gpsimd.dma_start(dst, src2[:, d_lo - 4 : d_hi - 4, :])
                if g < NG:
                    for cc in range(2):
                        fsrc = f1.ap()[128 * cc : 128 * (cc + 1), :]
                        sl = f1sb[cc][:][:, g * G * W : (g + 1) * G * W]
                        nc.gpsimd.dma_start(sl, fsrc[:, g * G * W : (g + 1) * G * W])
                        # pre-scale by 1/256 in place (exact in bf16)
                        nc.vector.tensor_scalar_mul(sl, sl, SCALE)

            # two persistent V buffers (manual double-buffering) so the
            # one-time pad memset [PATCH, VROW) is hazard-tracked; drains
            # never touch the pads so they stay zero across reuse
            vbufs = []
            for b in range(2):
                Vb = inp.tile([128, NTJ * VROW], BF16, tag=f"V{b}")
                vb = Vb[:]
                vbp = vb.ap[0][0]
                pad = AP(
                    vb.tensor,
                    vb.offset + PATCH,
                    [[vbp, 128], [VROW, NTJ], [1, VROW - PATCH]],
                )
                nc.vector.memset(pad, 0.0)
                vbufs.append(Vb)

            for _rep in range(repeat):
                for ti in range(NTI):
                    V = vbufs[ti % 2]
                    v = V[:]
                    vp = v.ap[0][0]
                    h0 = ti * TH
                    # 4 supertiles share one 4-bank PSUM tile (1 bank each);
                    # each supertile = 8 col-tiled matmuls (4 strips x 2 C)
                    for tj0 in range(0, NTJ, 4):
                        ng = min(4, NTJ - tj0)
                        T4 = ps.tile([128, 4 * BANK], F32, tag="T4")
                        t4 = T4[:]
                        tp = t4.ap[0][0]
                        for k in range(ng):
                            tj = tj0 + k
                            w0 = tj * TW
                            for cc in range(2):
                                a1 = f1sb[cc][:]
                                p1 = a1.ap[0][0]
                                a2 = f2sb[cc][:]
                                p2 = a2.ap[0][0]
                                for g in range(4):
                                    gy, gx = g >> 1, g & 1
                                    # f1 host-tiled: subtile's 32 pixels
                                    # contiguous at ((ti*NTJ+tj)*4+g)*32
                                    lhsT = AP(
                                        a1.tensor,
                                        a1.offset + ((ti * NTJ + tj) * 4 + g) * 32,
                                        [[p1, 128], [1, 32]],
                                    )
                                    rhs = AP(
                                        a2.tensor,
                                        a2.offset + (h0 + 4 * gy) * WP + w0 + 8 * gx,
                                        [[p2, 128], [WP, PH], [1, PW]],
                                    )
                                    out = AP(
                                        t4.tensor,
                                        t4.offset + 32 * g * tp + k * BANK,
                                        [[tp, 32], [1, PATCH]],
                                    )
                                    nc.tensor.matmul(
                                        out,
                                        lhsT,
                                        rhs,
                                        start=(cc == 0),
                                        stop=(cc == 1),
                                        tile_position=(0, 32 * g),
                                        skip_group_check=True,
                                    )
                        # grouped drains: ACT Prelu on cols [0,SA), DVE
                        # stt max(0.1x, x) drain on [SA,PATCH)
                        asrc = AP(t4.tensor, t4.offset, [[tp, 128], [BANK, ng], [1, SA]])
                        adst = AP(
                            v.tensor,
                            v.offset + tj0 * VROW,
                            [[vp, 128], [VROW, ng], [1, SA]],
                        )
                        if use_prelu:
                            nc.scalar.activation(
                                adst,
                                asrc,
                                mybir.ActivationFunctionType.Prelu,
                                bias=0.0,
                                scale=1.0,
                                alpha=0.1,
                            )
                        else:
                            nc.scalar.mul(adst, asrc, 1.0)
                        dsrc = AP(
                            t4.tensor, t4.offset + SA, [[tp, 128], [BANK, ng], [1, PATCH - SA]]
                        )
                        ddst = AP(
                            v.tensor,
                            v.offset + tj0 * VROW + SA,
                            [[vp, 128], [VROW, ng], [1, PATCH - SA]],
                        )
                        nc.vector.tensor_copy(ddst, dsrc)
                        nc.vector.scalar_tensor_tensor(
                            ddst,
                            ddst,
                            0.1,
                            ddst,
                            op0=mybir.AluOpType.mult,
                            op1=mybir.AluOpType.max,
                        )
                        if not use_prelu:
                            nc.vector.scalar_tensor_tensor(
                                adst,
                                adst,
                                0.1,
                                adst,
                                op0=mybir.AluOpType.mult,
                                op1=mybir.AluOpType.max,
                            )
                    # batched store: 10 padded patches -> O rows; alternate
                    # DGE queue (SP / Pool) per supertile row
                    odst = AP(
                        O.ap().tensor,
                        ti * NTJ * 128 * VROW,
                        [[VROW, 128], [128 * VROW, NTJ], [1, VROW]],
                    )
                    osrc3 = AP(v.tensor, v.offset, [[vp, 128], [VROW, NTJ], [1, VROW]])
                    if ti % 2 == 0:
                        nc.sync.dma_start(odst, osrc3)
                    else:
                        nc.gpsimd.dma_start(odst, osrc3)

    nc.compile()
    return nc


def _get_nc(repeat=1, use_prelu=True):
    key = ("nc", repeat, use_prelu)
    if key not in _cache:
        _cache[key] = _build(repeat, use_prelu)
    return _cache[key]


def _prep_f1(f1_sample):
    """[C, H, W] -> subtile-major [C, NTI*NTJ*4*32] so each subtile's 32
    pixels are contiguous (matmul weights need a single free dimension)."""
    t = f1_sample.reshape(C, NTI, 2, SH, NTJ, 2, SW)
    # order: ti, tj, g=(gy,gx), di, dj
    t = t.transpose(0, 1, 4, 2, 5, 3, 6)
    return np.ascontiguousarray(t.reshape(C, H * W), dtype=np.float32)


# host-side window gather indices: within a subtile patch [12, 16],
# pixel (di, dj) window value (dy, dx) lives at (di+dy)*16 + (dj+dx)
_DI = np.arange(SH)[:, None, None, None]
_DJ = np.arange(SW)[None, :, None, None]
_DY = np.arange(9)[None, None, :, None]
_DX = np.arange(9)[None, None, None, :]
_IDX = ((_DI + _DY) * PW + (_DJ + _DX)).reshape(SH, SW, 9, 9)


def _unpack(out_raw):
    """[n, NTI*NTJ*128, VROW] padded patches -> [n, 81, H, W] f32."""
    n = out_raw.shape[0]
    P = np.asarray(out_raw, dtype=np.float32).reshape(
        n, NTI, NTJ, 2, 2, SH * SW, VROW
    )
    # value[n, ti, tj, gy, gx, di, dj, dy, dx]
    Gt = P[:, :, :, :, :, np.arange(SH * SW)[:, None], _IDX.reshape(SH * SW, 81)]
    Gt = Gt.reshape(n, NTI, NTJ, 2, 2, SH, SW, 9, 9)
    # -> [n, dy, dx, ti, gy, di, tj, gx, dj]
    out = Gt.transpose(0, 7, 8, 1, 3, 5, 2, 4, 6).reshape(n, 81, H, W)
    return np.ascontiguousarray(out)


def _run(feat1, feat2, trace=False):
    from concourse.bass_utils import run_bass_kernel_spmd

    nc = _get_nc()
    in_maps = [
        {
            "f1": _prep_f1(feat1[i]),
            "f2": np.ascontiguousarray(feat2[i].reshape(C, H * W), dtype=np.float32),
        }
        for i in range(N)
    ]
    res = run_bass_kernel_spmd(nc, in_maps, core_ids=list(range(N)), trace=trace)
    out_raw = np.stack([np.asarray(res.results[i]["O"]) for i in range(N)])
    return _unpack(out_raw), res


def kernel(feat1, feat2):
    out, _ = _run(np.asarray(feat1), np.asarray(feat2))
    return out
